# revision 17
# baseline (speedup 1.0000x reference)
"""Multi-head attention (B=4, N=1568, C=768, H=12) on 8 TRN2 NeuronCores.

Sharding: query-parallel. Core c handles batch b = c // 2 and query half
half = c % 2 (784 query tokens). Each core computes K/V projections for the
full 1568 tokens of its batch (duplicated across the pair), Q projection
for its 784 tokens, full attention for all 12 heads over its queries, and
the output projection. No cross-core communication.

Host-side tricks:
  - tokens are rotated per core so its own query half sits at columns 0:784
    of xT; the key order is then a (core-dependent) permutation, which
    softmax attention is invariant to.
  - v_bias is folded into the projection bias:
      out = (attn + 1 (x) v_bias) @ proj_w + proj_b
          = attn @ proj_w + (proj_b + v_bias @ proj_w)
  - the softmax 1/sqrt(D) scale is folded into the exp activation's scale.
  - xT and wqk are pre-blocked on the host so every input DMA is a single
    contiguous read (xT by token chunk, wqk by 128-feature column block),
    letting the K/Q projections start as soon as their block lands.

Device schedule: a single software-pipelined stream over (head, key-tile)
steps. Each step emits the QK matmuls for the current tile, the exp
activation for the previous tile, the PV accumulation for a lagged tile
(so exp results buffer up while V is still being produced), and "filler"
projection units (V, later-feature K/Q, and partial output-projection
accumulated in SBUF) sized to keep the in-order PE queue fed without
starving the activation engine. The output projection is accumulated
j-block by j-block as heads complete, so only the last block + DMA remain
after the final head.
"""

import numpy as np
import ml_dtypes

B, N, C = 4, 1568, 768
H = 12
D = 64
NQ = N // 2          # 784 queries per core
SCALE = D ** -0.5
N_CORES = 8
KT = [128] * 12 + [32]            # key tiles (sum = 1568)
QCH = [(0, 512), (512, 272)]      # query chunks (PSUM-bank aligned)
TCH = [(0, 392), (392, 392), (784, 392), (1176, 392)]  # xT token chunks

_cache = {}


def _build_program():
    import concourse.mybir as mybir
    from concourse import bacc
    from concourse.tile import TileContext

    f32 = mybir.dt.float32
    f32r = mybir.dt.float32r
    bf16 = mybir.dt.bfloat16
    Exp = mybir.ActivationFunctionType.Exp
    Add = mybir.AluOpType.add

    nc = bacc.Bacc("TRN2", target_bir_lowering=False, debug=False,
                   num_devices=N_CORES)

    # host-blocked inputs: xTc[(c, j)] contiguous, wqk blocks contiguous
    xTc_d = nc.dram_tensor("xTc", [4 * 6 * 128, 392], bf16,
                           kind="ExternalInput")
    wqkb_d = nc.dram_tensor("wqkb", [12 * 6 * 128, 128], bf16,
                            kind="ExternalInput")
    wv_d = nc.dram_tensor("wv", [C, C], bf16, kind="ExternalInput")
    wp_d = nc.dram_tensor("wproj", [C, C], f32r, kind="ExternalInput")
    qb_d = nc.dram_tensor("qb", [128, 6], f32, kind="ExternalInput")
    pb_d = nc.dram_tensor("pb", [128, 6], f32, kind="ExternalInput")
    out_d = nc.dram_tensor("outT", [C, NQ], f32, kind="ExternalOutput")

    def xTc_blk(c, j):
        r = (c * 6 + j) * 128
        return xTc_d[r:r + 128, :]

    def wqkb_blk(g, j):
        r = (g * 6 + j) * 128
        return wqkb_d[r:r + 128, :]

    with TileContext(nc) as tc:
        persist_cm = tc.tile_pool(name="persist", bufs=1)
        persist = persist_cm.__enter__()
        kT = [persist.tile([128, N], bf16, tag=f"kT{j}", name=f"kT{j}")
              for j in range(6)]
        qT = [persist.tile([128, NQ], bf16, tag=f"qT{j}", name=f"qT{j}")
              for j in range(6)]
        v_sb = [persist.tile([128, H * (D + 1)], bf16, tag=f"v{t}",
                             name=f"v{t}") for t in range(13)]
        attn = [persist.tile([128, NQ], f32r, tag=f"at{j}", name=f"at{j}")
                for j in range(6)]
        acc = [persist.tile([128, NQ], f32, tag=f"acc{o}", name=f"acc{o}")
               for o in range(6)]
        qb_sb = persist.tile([128, 6], f32, tag="qb")
        pb_sb = persist.tile([128, 6], f32, tag="pb")
        nc.sync.dma_start(out=qb_sb, in_=qb_d[:])
        nc.sync.dma_start(out=pb_sb, in_=pb_d[:])

        wpp_cm = tc.tile_pool(name="wpp", bufs=1)
        wpp = wpp_cm.__enter__()
        wp_sb = [wpp.tile([128, C], f32r, tag=f"wp{j}", name=f"wp{j}")
                 for j in range(6)]

        phA_cm = tc.tile_pool(name="phA", bufs=1)
        phA = phA_cm.__enter__()
        xT = [phA.tile([128, N], bf16, tag=f"xT{j}", name=f"xTs{j}")
              for j in range(6)]
        # wqK[ft], wqQ[ft]: [128, 6*128]: column block ft of wqk, all six
        # 128-row input blocks side by side (lhsT slices for accumulation)
        wqK = [phA.tile([128, 6 * 128], bf16, tag=f"wqK{f}", name=f"wqK{f}")
               for f in range(6)]
        wqQ = [phA.tile([128, 6 * 128], bf16, tag=f"wqQ{f}", name=f"wqQ{f}")
               for f in range(6)]
        wv = [phA.tile([128, C], bf16, tag=f"wv{j}", name=f"wvs{j}")
              for j in range(6)]

        def dma_wq(ft):
            # K cols of ft live at blocked row group (6+ft), Q cols at
            # group ft (host packs Q blocks 0..5 then K blocks 6..11)
            for j in range(6):
                nc.sync.dma_start(out=wqK[ft][:, j * 128:(j + 1) * 128],
                                  in_=wqkb_blk(6 + ft, j))
                nc.sync.dma_start(out=wqQ[ft][:, j * 128:(j + 1) * 128],
                                  in_=wqkb_blk(ft, j))

        def dma_xt(c):
            t0, tw = TCH[c]
            for j in range(6):
                nc.sync.dma_start(out=xT[j][:, t0:t0 + tw],
                                  in_=xTc_blk(c, j))

        # DMA order = consumption order
        dma_wq(0)
        dma_xt(0)
        dma_xt(1)
        dma_wq(1)
        dma_xt(2)
        dma_xt(3)
        dma_wq(2)
        for j in range(6):
            nc.sync.dma_start(out=wv[j], in_=wv_d[j * 128:(j + 1) * 128, :])
        dma_wq(3)
        dma_wq(4)
        dma_wq(5)

        psA_cm = tc.tile_pool(name="psA", bufs=2, space="PSUM")
        psA = psA_cm.__enter__()
        psS_cm = tc.tile_pool(name="psS", bufs=2, space="PSUM")
        psS = psS_cm.__enter__()
        psO_cm = tc.tile_pool(name="psO", bufs=1, space="PSUM")
        psO = psO_cm.__enter__()
        exq_cm = tc.tile_pool(name="exq", bufs=18)
        exq = exq_cm.__enter__()
        phBn_cm = tc.tile_pool(name="phBn", bufs=2)
        phBn = phBn_cm.__enter__()

        # ---------------- projection / filler units ----------------
        def k_unit(ft, c):
            t0, tw = TCH[c]
            ps = psA.tile([128, 512], f32, tag="psA", name=f"k{ft}_{c}")
            for j in range(6):
                nc.tensor.matmul(ps[:, 0:tw], wqK[ft][:, j * 128:(j + 1) * 128],
                                 xT[j][:, t0:t0 + tw],
                                 start=(j == 0), stop=(j == 5))
            nc.vector.tensor_copy(kT[ft][:, t0:t0 + tw], ps[:, 0:tw])

        def q_unit(ft, qc):
            q0, qw = QCH[qc]
            ps = psA.tile([128, 512], f32, tag="psA", name=f"q{ft}_{qc}")
            for j in range(6):
                nc.tensor.matmul(ps[:, 0:qw], wqQ[ft][:, j * 128:(j + 1) * 128],
                                 xT[j][:, q0:q0 + qw],
                                 start=(j == 0), stop=(j == 5))
            nc.vector.tensor_scalar(out=qT[ft][:, q0:q0 + qw], in0=ps[:, 0:qw],
                                    scalar1=qb_sb[:, ft:ft + 1], scalar2=None,
                                    op0=Add)

        def v_unit(tt, vch):
            mt = KT[tt]
            v3 = v_sb[tt].rearrange("p (h e) -> p h e", h=H)
            ps = psA.tile([128, 512], f32, tag="psA", name=f"v{tt}_{vch}")
            for j in range(6):
                nc.tensor.matmul(ps[0:mt, 0:384],
                                 xT[j][:, tt * 128:tt * 128 + mt],
                                 wv[j][:, vch * 384:(vch + 1) * 384],
                                 start=(j == 0), stop=(j == 5))
            nc.vector.tensor_copy(
                v3[0:mt, vch * 6:(vch + 1) * 6, 0:64],
                ps[0:mt, 0:384].rearrange("p (h e) -> p h e", h=6))
            if vch == 1:
                nc.vector.memset(v3[0:mt, :, 64:65], 1.0)

        def wp_unit():
            for j in range(6):
                nc.sync.dma_start(out=wp_sb[j],
                                  in_=wp_d[j * 128:(j + 1) * 128, :])

        def pj_unit(j, ot):
            # partial output projection: acc[ot] (+)= wp[j].T @ attn[j]
            for (q0, qw) in QCH:
                ps = psA.tile([128, 512], f32, tag="psA", name=f"p{j}_{ot}")
                nc.tensor.matmul(ps[:, 0:qw], wp_sb[j][:, ot * 128:(ot + 1) * 128],
                                 attn[j][:, q0:q0 + qw], start=True, stop=True)
                if j == 0:
                    nc.vector.tensor_scalar(
                        out=acc[ot][:, q0:q0 + qw], in0=ps[:, 0:qw],
                        scalar1=pb_sb[:, ot:ot + 1], scalar2=None, op0=Add)
                else:
                    nc.vector.tensor_add(acc[ot][:, q0:q0 + qw],
                                         acc[ot][:, q0:q0 + qw], ps[:, 0:qw])
            if j == 5:
                nc.sync.dma_start(out=out_d[ot * 128:(ot + 1) * 128, :],
                                  in_=acc[ot])

        # unit list built lazily; each entry: (cost_ns, gate_fn, emit_fn)
        state = {"norm_done": 0, "v_done": 0}

        units = []
        # V tiles first (needed by PV of head 0), K/Q of ft f woven in so
        # kT/qT[f] are ready before head 2f begins (step 26*f).
        vlist = [(tt, vc) for tt in range(13) for vc in range(2)]
        kqlist = []
        for f in range(1, 6):
            kqlist += [("k", f, c) for c in range(4)]
            kqlist += [("q", f, qc) for qc in range(2)]
        inter = []
        vi, ki = 0, 0
        # pattern: 2 v units then 1 kq unit keeps both deadlines met
        while vi < len(vlist) or ki < len(kqlist):
            for _ in range(2):
                if vi < len(vlist):
                    inter.append(("v",) + vlist[vi]); vi += 1
            if ki < len(kqlist):
                inter.append(kqlist[ki]); ki += 1
        for u in inter:
            if u[0] == "v":
                _, tt, vc = u
                def em(tt=tt, vc=vc):
                    v_unit(tt, vc)
                    state["v_done"] = max(state["v_done"], 2 * tt + vc + 1)
                units.append((980, None, em))
            elif u[0] == "k":
                _, f, c = u
                units.append((1000, None, lambda f=f, c=c: k_unit(f, c)))
            else:
                _, f, qc = u
                units.append((520, None, lambda f=f, qc=qc: q_unit(f, qc)))
        units.append((200, None, wp_unit))
        for j in range(5):
            for ot in range(6):
                units.append(
                    (700,
                     (lambda j=j: state["norm_done"] >= 2 * j + 2),
                     lambda j=j, ot=ot: pj_unit(j, ot)))

        # ---------------- attention pipeline pieces ----------------
        def emit_qk(h, tt):
            ft, fo = h // 2, (h % 2) * 64
            mt = KT[tt]
            ps = psS.tile([128, 1024], f32, tag="psS", name=f"s{h}_{tt}")
            for (q0, qw) in QCH:
                nc.tensor.matmul(ps[0:mt, q0:q0 + qw],
                                 kT[ft][fo:fo + 64, tt * 128:tt * 128 + mt],
                                 qT[ft][fo:fo + 64, q0:q0 + qw],
                                 start=True, stop=True)
            return ps

        def emit_exp(h, tt, ps):
            mt = KT[tt]
            ex = exq.tile([128, NQ], bf16, tag="ex", name=f"ex{h}_{tt}")
            nc.scalar.activation(out=ex[0:mt, :], in_=ps[0:mt, 0:NQ],
                                 func=Exp, scale=SCALE)
            return ex

        po_of = {}

        def emit_pv(h, tt, ex):
            mt = KT[tt]
            if tt == 0:
                po_of[h] = psO.tile([65, 1024], f32, tag="psO", name=f"po{h}")
            po = po_of[h]
            vh = v_sb[tt].rearrange("p (h e) -> p h e", h=H)[0:mt, h, :]
            for (q0, qw) in QCH:
                nc.tensor.matmul(po[:, q0:q0 + qw], vh, ex[0:mt, q0:q0 + qw],
                                 start=(tt == 0), stop=(tt == 12))
            if tt == 12:
                emit_normalize(h, po)

        def emit_normalize(h, po):
            ft, fo = h // 2, (h % 2) * 64
            t65 = phBn.tile([65, NQ], f32, tag="t65", name=f"t65_{h}")
            nc.vector.tensor_copy(t65, po[:, 0:NQ])
            rec0 = phBn.tile([1, NQ], f32, tag="rec0", name=f"rc0_{h}")
            nc.gpsimd.dma_start(out=rec0, in_=t65[64:65, :])
            rec1 = phBn.tile([1, NQ], f32, tag="rec1", name=f"rc1_{h}")
            nc.vector.reciprocal_approx_fast(out=rec1, in_=rec0)
            rb = phBn.tile([64, NQ], f32, tag="rb", name=f"rb_{h}")
            nc.gpsimd.partition_broadcast(rb, rec1)
            stage = phBn.tile([64, NQ], f32r, tag="stage", name=f"st_{h}")
            nc.vector.tensor_mul(stage, t65[0:64, :], rb)
            nc.gpsimd.dma_start(out=attn[ft][fo:fo + 64, :], in_=stage)
            state["norm_done"] = h + 1

        # ---------------- prefix: K/Q for ft 0 ----------------
        with nc.named_scope("qkv"):
            for c in range(4):
                k_unit(0, c)
            q_unit(0, 0)
            q_unit(0, 1)

        # ---------------- pipelined main loop ----------------
        with nc.named_scope("attn"):
            steps = [(h, tt) for h in range(H) for tt in range(13)]
            qk_pend = None
            pv_q = []
            ui = 0                    # next unit index
            spent = 0.0               # ns of filler emitted
            budget = 0.0
            for si, (h, tt) in enumerate(steps):
                # front-load the filler budget while V/K/Q units remain
                # (PV is gated then, so the PE has the slack)
                budget += 900.0 if si < 44 else 520.0
                ps = emit_qk(h, tt)
                if qk_pend is not None:
                    ph, ptt, pps = qk_pend
                    ex = emit_exp(ph, ptt, pps)
                    pv_q.append((ph, ptt, ex))
                qk_pend = (h, tt, ps)
                # lagged PV: drain while backlog is deep or gate satisfied
                while pv_q:
                    vh_, vtt_, _ = pv_q[0]
                    if state["v_done"] < 2 * vtt_ + 2:
                        break          # V not produced yet (head 0 ramp)
                    if len(pv_q) <= 3 and vh_ == h:
                        break          # keep a small pipeline lag
                    emit_pv(*pv_q.pop(0))
                # fillers
                while ui < len(units) and spent < budget:
                    cost, gate, em = units[ui]
                    if gate is not None and not gate():
                        break
                    em()
                    spent += cost
                    ui += 1
            # drain
            ph, ptt, pps = qk_pend
            ex = emit_exp(ph, ptt, pps)
            pv_q.append((ph, ptt, ex))
            for item in pv_q:
                emit_pv(*item)
            while ui < len(units):
                cost, gate, em = units[ui]
                em()
                ui += 1

        # ---------------- tail: final projection block ----------------
        with nc.named_scope("proj"):
            for ot in range(6):
                pj_unit(5, ot)

        phBn_cm.__exit__(None, None, None)
        exq_cm.__exit__(None, None, None)
        psO_cm.__exit__(None, None, None)
        psS_cm.__exit__(None, None, None)
        psA_cm.__exit__(None, None, None)
        phA_cm.__exit__(None, None, None)
        wpp_cm.__exit__(None, None, None)
        persist_cm.__exit__(None, None, None)

    nc.compile()
    return nc


def _get_program():
    if "nc" not in _cache:
        _cache["nc"] = _build_program()
    return _cache["nc"]


def _make_in_maps(x, qkv_w, q_bias, v_bias, proj_w, proj_b):
    wv = np.ascontiguousarray(qkv_w[:, 2 * C:])       # [C, C]
    # wqk blocked: groups 0..5 = Q column blocks, 6..11 = K column blocks;
    # each group = [6 j-blocks, 128 rows, 128 cols] contiguous
    wqkb = np.empty((12, 6, 128, 128), np.float32)
    for f in range(6):
        for j in range(6):
            wqkb[f, j] = qkv_w[j * 128:(j + 1) * 128, f * 128:(f + 1) * 128]
            wqkb[6 + f, j] = qkv_w[j * 128:(j + 1) * 128,
                                   C + f * 128:C + (f + 1) * 128]
    wqkb = wqkb.reshape(12 * 6 * 128, 128).astype(ml_dtypes.bfloat16)

    qb = np.zeros((128, 6), np.float32)
    qb[:, :] = q_bias.reshape(6, 128).T
    pb_eff = proj_b + v_bias @ proj_w                  # fold v_bias into proj
    pb = np.zeros((128, 6), np.float32)
    pb[:, :] = pb_eff.reshape(6, 128).T

    in_maps = []
    for c in range(N_CORES):
        b, half = c // 2, c % 2
        xTr = np.roll(x[b].T, -half * NQ, axis=1)      # [C, N] rotated
        xtc = np.empty((4, 6, 128, 392), np.float32)
        for ci, (t0, tw) in enumerate(TCH):
            for j in range(6):
                xtc[ci, j] = xTr[j * 128:(j + 1) * 128, t0:t0 + tw]
        in_maps.append({
            "xTc": xtc.reshape(4 * 6 * 128, 392).astype(ml_dtypes.bfloat16),
            "wqkb": wqkb,
            "wv": wv.astype(ml_dtypes.bfloat16),
            "wproj": proj_w, "qb": qb, "pb": pb,
        })
    return in_maps


def kernel(x, qkv_w, q_bias, v_bias, proj_w, proj_b):
    from concourse.bass_utils import run_bass_kernel_spmd

    x = np.asarray(x, dtype=np.float32)
    qkv_w = np.asarray(qkv_w, dtype=np.float32)
    q_bias = np.asarray(q_bias, dtype=np.float32)
    v_bias = np.asarray(v_bias, dtype=np.float32)
    proj_w = np.asarray(proj_w, dtype=np.float32)
    proj_b = np.asarray(proj_b, dtype=np.float32)

    nc = _get_program()
    in_maps = _make_in_maps(x, qkv_w, q_bias, v_bias, proj_w, proj_b)
    _cache["in_maps"] = in_maps

    res = run_bass_kernel_spmd(nc, in_maps, list(range(N_CORES)))
    out = np.empty((B, N, C), np.float32)
    for c in range(N_CORES):
        b, half = c // 2, c % 2
        out[b, half * NQ:(half + 1) * NQ, :] = res.results[c]["outT"].T
    return out


# revision 26
# speedup vs baseline: 1.1165x; 1.1165x over previous
"""Multi-head attention (B=4, N=1568, C=768, H=12) on 8 TRN2 NeuronCores.

Sharding: query-parallel. Core c handles batch b = c // 2 and query half
half = c % 2 (784 query tokens). Each core computes K/V projections for the
full 1568 tokens of its batch (duplicated across the pair), Q projection
for its 784 tokens, full attention for all 12 heads over its queries, and
the output projection. No cross-core communication.

Host-side tricks:
  - tokens are rotated per core so its own query half sits at columns 0:784
    of xT; the key order is then a (core-dependent) permutation, which
    softmax attention is invariant to. This removes the separate xqT input.
  - v_bias is folded into the projection bias:
      out = (attn + 1 (x) v_bias) @ proj_w + proj_b
          = attn @ proj_w + (proj_b + v_bias @ proj_w)
  - the softmax 1/sqrt(D) scale is folded into the exp activation's scale.

Device layouts (per core):
  xT   [768, 1568]  x[b].T rotated       (feature-major)
  K^T  [768, 1568]  feature-major K
  Q^T  [768, 784]   feature-major Q (+q_bias)
  V    [1568, 780]  token-major V as 12 heads x (64 cols + ones col)
  scores^T [keys, q] per (head, key-tile) in PSUM -> exp -> bf16 SBUF
  PV   -> psum [65, 784]: rows 0-63 = (expS @ V)^T, row 64 = softmax denom
  attn^T [768, 784]  normalized attention, feature-major
  outT [768, 784]   proj_w.T @ attn^T + (proj_b + v_bias @ proj_w)
"""

import numpy as np
import ml_dtypes

B, N, C = 4, 1568, 768
H = 12
D = 64
NQ = N // 2          # 784 queries per core
SCALE = D ** -0.5
N_CORES = 8
KT = [128] * 12 + [32]          # key tiles (sum = 1568)
QCH = [(0, 512), (512, 272)]    # query chunks (PSUM-bank aligned)
TCH = [(0, 392), (392, 392), (784, 392), (1176, 392)]  # token chunks (phase A)

_cache = {}


def _build_program():
    import concourse.mybir as mybir
    from concourse import bacc
    from concourse.tile import TileContext

    f32 = mybir.dt.float32
    f32r = mybir.dt.float32r
    bf16 = mybir.dt.bfloat16
    Exp = mybir.ActivationFunctionType.Exp

    nc = bacc.Bacc("TRN2", target_bir_lowering=False, debug=False,
                   num_devices=N_CORES)

    xT_d = nc.dram_tensor("xT", [C, N], bf16, kind="ExternalInput")
    # wqk re-blocked on host: row group ft (128 rows) holds, per partition,
    # the 6 K j-blocks then the 6 Q j-blocks of column block ft, so one
    # contiguous 0.39 MB DMA delivers everything feature-block ft needs.
    wqkb_d = nc.dram_tensor("wqkb", [C, 12 * 128], bf16, kind="ExternalInput")
    wv_d = nc.dram_tensor("wv", [C, C], bf16, kind="ExternalInput")
    wp_d = nc.dram_tensor("wproj", [C, C], f32r, kind="ExternalInput")
    qb_d = nc.dram_tensor("qb", [128, 6], f32, kind="ExternalInput")
    pb_d = nc.dram_tensor("pb", [128, 6], f32, kind="ExternalInput")
    out_d = nc.dram_tensor("outT", [C, NQ], f32, kind="ExternalOutput")

    with TileContext(nc) as tc:
        persist_cm = tc.tile_pool(name="persist", bufs=1)
        persist = persist_cm.__enter__()
        kT = [persist.tile([128, N], bf16, tag=f"kT{j}", name=f"kT{j}")
              for j in range(6)]
        qT = [persist.tile([128, NQ], bf16, tag=f"qT{j}", name=f"qT{j}")
              for j in range(6)]
        v_sb = [persist.tile([128, H * (D + 1)], bf16, tag=f"v{t}", name=f"v{t}")
                for t in range(13)]
        attn = [persist.tile([128, NQ], f32r, tag=f"at{j}", name=f"at{j}")
                for j in range(6)]
        acc = [persist.tile([128, NQ], f32, tag=f"acc{o}", name=f"acc{o}")
               for o in range(6)]
        qb_sb = persist.tile([128, 6], f32, tag="qb")
        pb_sb = persist.tile([128, 6], f32, tag="pb")
        nc.sync.dma_start(out=qb_sb, in_=qb_d[:])
        nc.sync.dma_start(out=pb_sb, in_=pb_d[:])

        wpp_cm = tc.tile_pool(name="wpp", bufs=1)
        wpp = wpp_cm.__enter__()
        wp_sb = [wpp.tile([128, C], f32r, tag=f"wp{j}", name=f"wp{j}")
                 for j in range(6)]

        # PE warm-up source tile (junk): memset now, matmuls emitted after the
        # input DMAs are issued so the HAM clock gate opens (1.2 -> 2.4 GHz)
        # during the DMA prologue without gating anything behind a pool
        # barrier.
        jw = persist.tile([128, 512], bf16, tag="jw")
        nc.vector.memset(jw, 0.0)

        # ========== phases A+B merged: QKV projections + attention ==========
        # One PSUM layout for both: psA (1 bank x 2) for projections,
        # psS (2 banks x 2) for scores, psO (2 banks x 1) for PV accum.
        phA_cm = tc.tile_pool(name="phA", bufs=1)
        phA = phA_cm.__enter__()
        xT = [phA.tile([128, N], bf16, tag=f"xT{j}", name=f"xTs{j}")
              for j in range(6)]
        # wq_sb[ft]: [128, 1536] = 6 K j-blocks then 6 Q j-blocks of
        # wqk's column block ft
        wq_sb = [phA.tile([128, 12 * 128], bf16, tag=f"wq{f}", name=f"wqs{f}")
                 for f in range(6)]
        wv = [phA.tile([128, C], bf16, tag=f"wv{j}", name=f"wvs{j}")
              for j in range(6)]
        # DMA order = consumption order: ft0 K+Q weights, then x, then wv
        # (head 0's V), then the remaining weight blocks.
        nc.sync.dma_start(out=wq_sb[0], in_=wqkb_d[0:128, :])
        for j in range(6):
            nc.sync.dma_start(out=xT[j], in_=xT_d[j * 128:(j + 1) * 128, :])
        for j in range(6):
            nc.sync.dma_start(out=wv[j], in_=wv_d[j * 128:(j + 1) * 128, :])
        for f in range(1, 6):
            nc.sync.dma_start(out=wq_sb[f], in_=wqkb_d[f * 128:(f + 1) * 128, :])

        psA_cm = tc.tile_pool(name="psA", bufs=2, space="PSUM")
        psA = psA_cm.__enter__()
        # warm-up junk matmuls (overlap the input DMA; results never read)
        for wi in range(14):
            psw = psA.tile([128, 512], f32, tag="psA", name=f"warm{wi}")
            nc.tensor.matmul(psw, jw[:, 0:128], jw, start=True, stop=True)
        psS_cm = tc.tile_pool(name="psS", bufs=2, space="PSUM")
        psS = psS_cm.__enter__()
        psO_cm = tc.tile_pool(name="psO", bufs=1, space="PSUM")
        psO = psO_cm.__enter__()
        phB_cm = tc.tile_pool(name="phB", bufs=3)
        phB = phB_cm.__enter__()
        phBn_cm = tc.tile_pool(name="phBn", bufs=2)
        phBn = phBn_cm.__enter__()

        def emit_k(ft):
            for (t0, tw) in TCH:
                ps = psA.tile([128, 512], f32, tag="psA", name=f"k{ft}_{t0}")
                for j in range(6):
                    nc.tensor.matmul(
                        ps[:, 0:tw],
                        wq_sb[ft][:, j * 128:(j + 1) * 128],
                        xT[j][:, t0:t0 + tw],
                        start=(j == 0), stop=(j == 5),
                    )
                nc.vector.tensor_copy(kT[ft][:, t0:t0 + tw], ps[:, 0:tw])

        def emit_q(ft):
            for (t0, tw) in TCH[:2]:
                ps = psA.tile([128, 512], f32, tag="psA", name=f"q{ft}_{t0}")
                for j in range(6):
                    nc.tensor.matmul(
                        ps[:, 0:tw],
                        wq_sb[ft][:, 768 + j * 128:768 + (j + 1) * 128],
                        xT[j][:, t0:t0 + tw],
                        start=(j == 0), stop=(j == 5),
                    )
                nc.vector.tensor_scalar(
                    out=qT[ft][:, t0:t0 + tw], in0=ps[:, 0:tw],
                    scalar1=qb_sb[:, ft:ft + 1], scalar2=None,
                    op0=mybir.AluOpType.add,
                )

        def pj_unit(j, ot):
            # partial output projection: acc[ot] (+)= wp[j].T @ attn[j]
            # (bias folded into the j == 0 copy); j == 5 also streams the
            # finished block to DRAM
            for (q0, qw) in QCH:
                ps = psA.tile([128, 512], f32, tag="psA", name=f"p{j}_{ot}")
                nc.tensor.matmul(
                    ps[:, 0:qw], wp_sb[j][:, ot * 128:(ot + 1) * 128],
                    attn[j][:, q0:q0 + qw], start=True, stop=True)
                if j == 0:
                    nc.vector.tensor_scalar(
                        out=acc[ot][:, q0:q0 + qw], in0=ps[:, 0:qw],
                        scalar1=pb_sb[:, ot:ot + 1], scalar2=None,
                        op0=mybir.AluOpType.add)
                else:
                    nc.vector.tensor_add(acc[ot][:, q0:q0 + qw],
                                         acc[ot][:, q0:q0 + qw], ps[:, 0:qw])
            if j == 5:
                nc.sync.dma_start(out=out_d[ot * 128:(ot + 1) * 128, :],
                                  in_=acc[ot])

        def emit_v(tt):
            mt = KT[tt]
            v3 = v_sb[tt].rearrange("p (h e) -> p h e", h=H)
            for vch in range(2):
                ps = psA.tile([128, 512], f32, tag="psA", name=f"v{tt}_{vch}")
                for j in range(6):
                    nc.tensor.matmul(
                        ps[0:mt, 0:384],
                        xT[j][:, tt * 128:tt * 128 + mt],
                        wv[j][:, vch * 384:(vch + 1) * 384],
                        start=(j == 0), stop=(j == 5),
                    )
                nc.vector.tensor_copy(
                    v3[0:mt, vch * 6:(vch + 1) * 6, 0:64],
                    ps[0:mt, 0:384].rearrange("p (h e) -> p h e", h=6),
                )
            nc.vector.memset(v3[0:mt, :, 64:65], 1.0)

        with nc.named_scope("qkv"):
            emit_k(0)
            emit_q(0)

        with nc.named_scope("attn"):
            po_of = {}

            def emit_qk(h, tt):
                ft, fo = h // 2, (h % 2) * 64
                mt = KT[tt]
                ps = psS.tile([128, 1024], f32, tag="psS", name=f"s{h}_{tt}")
                for (q0, qw) in QCH:
                    nc.tensor.matmul(
                        ps[0:mt, q0:q0 + qw],
                        kT[ft][fo:fo + 64, tt * 128:tt * 128 + mt],
                        qT[ft][fo:fo + 64, q0:q0 + qw],
                        start=True, stop=True,
                    )
                return ps

            def emit_exp_pv(h, tt, ps):
                mt = KT[tt]
                if tt == 0:
                    po_of[h] = psO.tile([65, 1024], f32, tag="psO",
                                        name=f"po{h}")
                po = po_of[h]
                ex = phB.tile([128, NQ], bf16, tag="ex", name=f"ex{h}_{tt}")
                nc.scalar.activation(out=ex[0:mt, :], in_=ps[0:mt, 0:NQ],
                                     func=Exp, scale=SCALE)
                vh = v_sb[tt].rearrange("p (h e) -> p h e", h=H)[0:mt, h, :]
                for (q0, qw) in QCH:
                    nc.tensor.matmul(
                        po[:, q0:q0 + qw],
                        vh,
                        ex[0:mt, q0:q0 + qw],
                        start=(tt == 0), stop=(tt == 12),
                    )
                if tt == 12:
                    emit_normalize(h, po)

            def emit_normalize(h, po):
                # rows 0-63 / row 64 (denominator). Custom DVE / gpsimd ops
                # only work from partition 0, so evict PSUM to SBUF,
                # DMA-shift the denominator row to partition 0, then
                # recip+broadcast+multiply there.
                ft, fo = h // 2, (h % 2) * 64
                t65 = phBn.tile([65, NQ], f32, tag="t65", name=f"t65_{h}")
                nc.vector.tensor_copy(t65, po[:, 0:NQ])
                rec0 = phBn.tile([1, NQ], f32, tag="rec0", name=f"rc0_{h}")
                nc.gpsimd.dma_start(out=rec0, in_=t65[64:65, :])
                rec1 = phBn.tile([1, NQ], f32, tag="rec1", name=f"rc1_{h}")
                nc.vector.reciprocal_approx_fast(out=rec1, in_=rec0)
                rb = phBn.tile([64, NQ], f32, tag="rb", name=f"rb_{h}")
                nc.gpsimd.partition_broadcast(rb, rec1)
                stage = phBn.tile([64, NQ], f32r, tag="stage", name=f"st_{h}")
                nc.vector.tensor_mul(stage, t65[0:64, :], rb)
                nc.gpsimd.dma_start(out=attn[ft][fo:fo + 64, :], in_=stage)

            pend = None
            for h in range(H):
                for tt in range(13):
                    if h == 0:
                        emit_v(tt)          # V tiles stream in under head 0
                    ps = emit_qk(h, tt)
                    if pend is not None:
                        emit_exp_pv(*pend)
                    pend = (h, tt, ps)
                # interleave remaining K/Q projection blocks, the wproj
                # load, and partial output-projection accumulation (for
                # feature blocks whose heads are normalized) into the
                # attention stream as PE gap fillers
                if h in (1, 3, 5, 7, 9):
                    emit_k(h // 2 + 1)
                    emit_q(h // 2 + 1)
                if h == 1:
                    for j in range(6):
                        nc.sync.dma_start(
                            out=wp_sb[j],
                            in_=wp_d[j * 128:(j + 1) * 128, :])
                if h in (3, 5, 7, 9, 11):
                    jp = (h - 3) // 2       # attn[jp] done after head 2jp+1
                    for ot in range(6):
                        pj_unit(jp, ot)
            emit_exp_pv(*pend)

        # ============ tail: last projection block + output DMA ============
        with nc.named_scope("proj"):
            for ot in range(6):
                pj_unit(5, ot)

        phBn_cm.__exit__(None, None, None)
        phB_cm.__exit__(None, None, None)
        psO_cm.__exit__(None, None, None)
        psS_cm.__exit__(None, None, None)
        psA_cm.__exit__(None, None, None)
        phA_cm.__exit__(None, None, None)
        wpp_cm.__exit__(None, None, None)
        persist_cm.__exit__(None, None, None)

    nc.compile()
    return nc


def _get_program():
    if "nc" not in _cache:
        _cache["nc"] = _build_program()
    return _cache["nc"]


def _make_in_maps(x, qkv_w, q_bias, v_bias, proj_w, proj_b):
    wv = np.ascontiguousarray(qkv_w[:, 2 * C:])       # [C, C]
    # wqkb[ft, p, b, c]: b<6 -> K j-block b, b>=6 -> Q j-block (b-6), of
    # wqk's 128-wide column block ft; one contiguous read per ft group
    wqkb = np.empty((6, 128, 12, 128), np.float32)
    for f in range(6):
        for j in range(6):
            wqkb[f, :, j, :] = qkv_w[j * 128:(j + 1) * 128,
                                     C + f * 128:C + (f + 1) * 128]
            wqkb[f, :, 6 + j, :] = qkv_w[j * 128:(j + 1) * 128,
                                         f * 128:(f + 1) * 128]
    wqkb = wqkb.reshape(C, 12 * 128).astype(ml_dtypes.bfloat16)
    qb = np.zeros((128, 6), np.float32)
    qb[:, :] = q_bias.reshape(6, 128).T
    pb_eff = proj_b + v_bias @ proj_w                  # fold v_bias into proj
    pb = np.zeros((128, 6), np.float32)
    pb[:, :] = pb_eff.reshape(6, 128).T

    in_maps = []
    for c in range(N_CORES):
        b, half = c // 2, c % 2
        # rotate tokens so this core's query half sits at columns 0:NQ;
        # key order becomes a permutation, which softmax attention is
        # invariant to
        xT = np.ascontiguousarray(
            np.roll(x[b].T, -half * NQ, axis=1)).astype(ml_dtypes.bfloat16)
        in_maps.append({
            "xT": xT, "wqkb": wqkb,
            "wv": wv.astype(ml_dtypes.bfloat16),
            "wproj": proj_w, "qb": qb, "pb": pb,
        })
    return in_maps


def kernel(x, qkv_w, q_bias, v_bias, proj_w, proj_b):
    from concourse.bass_utils import run_bass_kernel_spmd

    x = np.asarray(x, dtype=np.float32)
    qkv_w = np.asarray(qkv_w, dtype=np.float32)
    q_bias = np.asarray(q_bias, dtype=np.float32)
    v_bias = np.asarray(v_bias, dtype=np.float32)
    proj_w = np.asarray(proj_w, dtype=np.float32)
    proj_b = np.asarray(proj_b, dtype=np.float32)

    nc = _get_program()
    in_maps = _make_in_maps(x, qkv_w, q_bias, v_bias, proj_w, proj_b)
    _cache["in_maps"] = in_maps

    res = run_bass_kernel_spmd(nc, in_maps, list(range(N_CORES)))
    out = np.empty((B, N, C), np.float32)
    for c in range(N_CORES):
        b, half = c // 2, c % 2
        out[b, half * NQ:(half + 1) * NQ, :] = res.results[c]["outT"].T
    return out

